# revision 25
# baseline (speedup 1.0000x reference)
"""Trainium2 Bass kernel for nn_Decoder (LSTM decoder: embed -> LSTM -> vocab fc).

Strategy (v2):
  - Host folds embedding + input projection + biases into one gather table:
    xg_table = embed_W @ W_ih^T + b_ih + b_hh  (gate-column-permuted). Per
    step the kernel indirect-DMA-gathers 16 rows -> no device-side embedding
    transposes, no xg matmuls, no bias matmuls.
  - Recurrence is data-parallel over batch (16 rows/core), 4-way PE column
    tiling for the h @ W_hh^T matmul (as before). h^T is produced by 2 PE
    transposes + 8 narrow DVE copies (instead of 32 DVE transposes).
  - Every S=8 steps, the per-core h^T block is AllGather'd (DRAM->DRAM) so
    every core holds h^T for the FULL batch; the fc projection is sharded
    over vocab (1280 rows/core) with its weights RESIDENT in SBUF, and its
    matmuls (full-array, N=512) are interleaved into the recurrence's PE
    gaps.
"""

import sys

sys.path.insert(0, "/opt/trn_rl_repo")

import numpy as np
import ml_dtypes

import concourse.bass as bass
import concourse.bacc as bacc
import concourse.mybir as mybir
import concourse.tile as tile

BF16 = ml_dtypes.bfloat16

# Problem shapes
B, T, E, H, V = 128, 64, 512, 1024, 10000
NCORES = 8
BC = B // NCORES        # 16 batch rows per core
G = 4                   # PE column-tile groups for the recurrence
S = 8                   # steps per h^T block (AllGather granularity)
NB = T // S             # 8 blocks
VC = 1280               # vocab rows per core (10240 padded / 8)
NVT = VC // 128         # 10 vocab tiles per core
VAUG = V + BC           # xg table rows: vocab + per-core feature rows
LAG = 3                 # steps between block boundary and fc eligibility
FC_A = 1                # fc units emitted after the h-matmuls
FC_B = 1                # fc units emitted after the transposes

F32 = mybir.dt.float32
BF = mybir.dt.bfloat16
I32 = mybir.dt.int32


def build_nc():
    nc = bacc.Bacc("TRN2", num_devices=NCORES)

    xgt_d = nc.declare_dram_parameter("xgt", [VAUG, 4 * H], BF, isOutput=False)
    idx_d = nc.declare_dram_parameter("idx", [BC, T], I32, isOutput=False)
    whh_d = nc.declare_dram_parameter("whh", [G, 8, 128, 1024], BF, isOutput=False)
    sel_d = nc.declare_dram_parameter("sel16", [128, BC], BF, isOutput=False)
    id_d = nc.declare_dram_parameter("ident", [128, 128], BF, isOutput=False)
    fcw_d = nc.declare_dram_parameter("fcw", [NVT, 8, 128, 128], BF, isOutput=False)
    fcb_d = nc.declare_dram_parameter("fcb", [128, NVT], F32, isOutput=False)
    # out layout: (vtile, vpart, src_core, block, s*16+j)
    out_d = nc.declare_dram_parameter(
        "out_lg", [NVT, 128, NCORES, NB, S * BC], F32, isOutput=True
    )

    # partition-major staging so gathered blocks load with one DMA per core
    hsb_in_d = nc.dram_tensor("hsb_in", [NB, 128, 8, S * BC], BF)
    hsb_out_d = nc.dram_tensor(
        "hsb_out", [NB, NCORES, 128, 8, S * BC], BF, addr_space="Shared"
    )

    XB = 4  # xgm prefetch depth

    with tile.TileContext(nc) as tc:
        with (
            tc.tile_pool(name="persist", bufs=1) as pp,
            tc.tile_pool(name="gates", bufs=3) as gates_p,
            tc.tile_pool(name="ew", bufs=3) as ew_p,
            tc.tile_pool(name="logit", bufs=6) as logit_p,
            tc.tile_pool(name="gpsum", bufs=1, space="PSUM") as gps_p,
            tc.tile_pool(name="fpsum", bufs=3, space="PSUM") as fps_p,
            tc.tile_pool(name="tpsum", bufs=1, space="PSUM") as tps_p,
        ):
            # ---- small persistent tiles first (cheap DMAs, unblock step 0) ----
            idx_sb = pp.tile([BC, T], I32, tag="idx")
            nc.sync.dma_start(out=idx_sb[:, :], in_=idx_d[:, :])
            sel_sb = pp.tile([128, BC], BF, tag="sel16")
            nc.sync.dma_start(out=sel_sb[:, :], in_=sel_d[:, :])
            id_sb = pp.tile([128, 128], BF, tag="ident")
            nc.sync.dma_start(out=id_sb[:, :], in_=id_d[:, :])
            fcb_sb = pp.tile([128, NVT], F32, tag="fcb")
            nc.sync.dma_start(out=fcb_sb[:, :], in_=fcb_d[:, :])

            # xgm ring: rows 0:16 hold gathered xg rows; junk rows stay 0
            xgm = []
            for par in range(XB):
                xt = pp.tile([128, 4 * H], BF, tag=f"xgm_{par}", name=f"xgm_{par}")
                nc.vector.memset(xt[:, :], 0.0)
                xgm.append(xt)

            def gather_xg(t):
                nc.gpsimd.indirect_dma_start(
                    out=xgm[t % XB][0:BC, :],
                    out_offset=None,
                    in_=xgt_d[:, :],
                    in_offset=bass.IndirectOffsetOnAxis(
                        ap=idx_sb[:, t : t + 1], axis=0
                    ),
                )

            for t in range(min(XB - 1, T)):
                gather_xg(t)

            # ---- weights ----
            whh_sb = {}
            for k in range(8):
                for g in range(G):
                    w = pp.tile([128, 1024], BF, tag=f"w_{g}_{k}", name=f"w_{g}_{k}")
                    nc.sync.dma_start(out=w[:, :], in_=whh_d[g, k, :, :])
                    whh_sb[(g, k)] = w
            fcw_sb = []
            for v in range(NVT):
                fw = pp.tile([128, 1024], BF, tag=f"fcw_{v}", name=f"fcw_{v}")
                for kc in range(8):
                    nc.sync.dma_start(
                        out=fw[:, 128 * kc : 128 * kc + 128], in_=fcw_d[v, kc, :, :]
                    )
                fcw_sb.append(fw)

            # ---- state tiles ----
            # h^T accumulation ring: 2 block slots of S*BC=128 cols per chunk
            hsT = [
                pp.tile([128, 2 * S * BC], BF, tag=f"hsT_{kc}", name=f"hsT_{kc}")
                for kc in range(8)
            ]
            # gathered full-batch h^T ring: 2 block slots, kc-major columns
            hfull = [
                pp.tile([128, 8, S * B], BF, tag=f"hf_{sl}", name=f"hf_{sl}")
                for sl in range(2)
            ]
            c_sb = pp.tile([128, 256], F32, tag="c_state")

            gps = [
                gps_p.tile([128, 1024], F32, tag="gps0", name="gps0"),
                gps_p.tile([128, 1024], F32, tag="gps1", name="gps1"),
            ]
            nc.vector.memset(gps[0][:, :], 0.0)
            nc.vector.memset(gps[1][:, :], 0.0)

            # ---- fc emission ----
            fc_queue = []  # (block, vtile, half) eligible units

            evict_list = []  # fc units whose matmuls are issued, eviction pending

            def emit_fc_mms(n):
                """Issue the PE matmuls for up to n queued fc units (one unit =
                both 512-token halves of a (block, vtile), sharing each
                stationary load); defer the ACT eviction so it never precedes
                the step's gate activations in the in-order ACT queue."""
                for _ in range(min(n, len(fc_queue))):
                    b, v = fc_queue.pop(0)
                    sl = b % 2
                    fps = [
                        fps_p.tile([128, 512], F32, tag="fps", name=f"fps{i}")
                        for i in range(2)
                    ]
                    for kc in range(8):
                        for hf in range(2):
                            nc.tensor.matmul(
                                fps[hf][:, :],
                                fcw_sb[v][:, 128 * kc : 128 * kc + 128],
                                hfull[sl][:, kc, 512 * hf : 512 * hf + 512],
                                start=(kc == 0),
                                stop=(kc == 7),
                                skip_group_check=True,
                            )
                    for hf in range(2):
                        evict_list.append((fps[hf], b, v, hf))

            def flush_evicts():
                while evict_list:
                    fps, b, v, hf = evict_list.pop(0)
                    lg = logit_p.tile([128, 512], F32, tag="lg")
                    nc.scalar.activation(
                        lg[:, :],
                        fps[:, :],
                        mybir.ActivationFunctionType.Identity,
                        bias=fcb_sb[:, v : v + 1],
                    )
                    nc.sync.dma_start(
                        out=out_d[v, :, 4 * hf : 4 * hf + 4, b, :], in_=lg[:, :]
                    )

            pending = []  # blocks gathered but not yet eligible: (block, ready_t)

            def release_pending(t):
                while pending and pending[0][1] <= t:
                    b, _ = pending.pop(0)
                    for v in range(NVT):
                        fc_queue.append((b, v))

            # ---- recurrence ----
            for t in range(T):
                ps = gps[t % 2]
                release_pending(t)
                if t + XB - 1 < T:
                    gather_xg(t + XB - 1)

                # gate matmuls: xg injection first (independent of h(t-1)),
                # then the 8 h-chunk contributions
                nks = 1 if t == 0 else 9
                for half in range(2):
                    cs = slice(512 * half, 512 * half + 512)
                    for g in range(G):
                        nc.tensor.matmul(
                            ps[32 * g : 32 * g + BC, cs],
                            sel_sb[:, :],
                            xgm[t % XB][:, 1024 * g + 512 * half :][:, 0:512],
                            start=True,
                            stop=(nks == 1),
                            tile_position=(0, 32 * g),
                            skip_group_check=True,
                        )
                if t > 0:
                    pc = ((t - 1) // S) % 2 * (S * BC) + ((t - 1) % S) * BC
                    for ki in range(8):
                        for half in range(2):
                            cs = slice(512 * half, 512 * half + 512)
                            for g in range(G):
                                nc.tensor.matmul(
                                    ps[32 * g : 32 * g + BC, cs],
                                    hsT[ki][:, pc : pc + BC],
                                    whh_sb[(g, ki)][:, cs],
                                    start=False,
                                    stop=(ki == 7),
                                    tile_position=(0, 32 * g),
                                    skip_group_check=True,
                                )

                emit_fc_mms(FC_A)

                # elementwise: gate order per group is [i | f | o | g]
                gt = gates_p.tile([128, 1024], F32, tag="gt")
                nc.scalar.activation(
                    gt[:, 0:768], ps[:, 0:768], mybir.ActivationFunctionType.Sigmoid
                )
                nc.scalar.activation(
                    gt[:, 768:1024], ps[:, 768:1024], mybir.ActivationFunctionType.Tanh
                )
                if t == 0:
                    nc.vector.tensor_mul(c_sb[:, :], gt[:, 0:256], gt[:, 768:1024])
                else:
                    tmp1 = ew_p.tile([128, 256], F32, tag="tmp1")
                    nc.vector.tensor_mul(tmp1[:, :], gt[:, 0:256], gt[:, 768:1024])
                    nc.vector.tensor_mul(c_sb[:, :], gt[:, 256:512], c_sb[:, :])
                    nc.vector.tensor_add(c_sb[:, :], c_sb[:, :], tmp1[:, :])
                tcs = ew_p.tile([128, 256], F32, tag="tcs")
                nc.scalar.activation(
                    tcs[:, :], c_sb[:, :], mybir.ActivationFunctionType.Tanh
                )
                h_sb = ew_p.tile([128, 256], BF, tag="h_sb")
                nc.vector.tensor_mul(h_sb[:, :], gt[:, 512:768], tcs[:, :])

                flush_evicts()

                # h -> h^T: 2 PE transposes + 8 narrow copies
                cc = (t // S) % 2 * (S * BC) + (t % S) * BC
                for gam in range(2):
                    tps = tps_p.tile([128, 128], BF, tag="tps")
                    nc.tensor.transpose(
                        tps[:, :], h_sb[:, 128 * gam : 128 * gam + 128], id_sb[:, :]
                    )
                    for g in range(G):
                        nc.vector.tensor_copy(
                            hsT[2 * g + gam][:, cc : cc + BC],
                            tps[:, 32 * g : 32 * g + BC],
                        )

                # ~1.5 units/step matches the 1.25/step inflow, so fc work
                # spreads across all steps instead of bursting after releases
                emit_fc_mms(FC_B if t % 2 == 0 else 0)
                flush_evicts()

                # block boundary: stage own h^T block, AllGather, load gathered
                if (t + 1) % S == 0:
                    b = t // S
                    sl = b % 2
                    for kc in range(8):
                        nc.sync.dma_start(
                            out=hsb_in_d[b, :, kc, :],
                            in_=hsT[kc][:, sl * S * BC : (sl + 1) * S * BC],
                        )
                    nc.gpsimd.collective_compute(
                        "AllGather",
                        mybir.AluOpType.bypass,
                        replica_groups=[list(range(NCORES))],
                        ins=[hsb_in_d[b, :, :, :]],
                        outs=[hsb_out_d[b, :, :, :, :]],
                    )
                    for core in range(NCORES):
                        nc.gpsimd.dma_start(
                            out=hfull[sl][:, :, 128 * core : 128 * core + 128],
                            in_=hsb_out_d[b, core, :, :, :],
                        )
                    pending.append((b, t + 1 + LAG))

            # ---- epilogue: drain remaining fc work ----
            release_pending(10**9)
            while fc_queue:
                emit_fc_mms(1)
                flush_evicts()

    nc.finalize()
    return nc


def prep_host(features, captions, embed_W, W_ih, W_hh, b_ih, b_hh, fc_W, fc_b):
    """Host-side layout prep. Returns (shared dict, per-core list)."""
    # gate-column permutation: group g holds H-range [256g:256g+256) of each
    # gate, column order within group = [i | f | o | gg] (256 each)
    sec_base = np.array([0, H, 3 * H, 2 * H])
    j = np.arange(1024)
    perm = np.empty((G, 1024), np.int64)
    for g in range(G):
        perm[g] = sec_base[j // 256] + 256 * g + (j % 256)
    full_perm = perm.reshape(-1)  # [4096] column order: group-major

    bias = (b_ih + b_hh).astype(np.float32)

    # xg gather table: (embed @ W_ih^T + bias), columns permuted
    xgt_core = (embed_W.astype(np.float32) @ W_ih.T.astype(np.float32)) + bias
    xgt_core = xgt_core[:, full_perm].astype(BF16)  # [V, 4096]
    feat_xg = (features.astype(np.float32) @ W_ih.T.astype(np.float32)) + bias
    feat_xg = feat_xg[:, full_perm].astype(BF16)  # [B, 4096]

    whh = np.zeros((G, 8, 128, 1024), np.float32)
    for g in range(G):
        selw = W_hh[perm[g]]  # [1024 gate-cols, 1024]
        for k in range(8):
            whh[g, k] = selw[:, 128 * k : 128 * k + 128].T
    whh = whh.astype(BF16)

    sel16 = np.zeros((128, BC), np.float32)
    sel16[:BC, :BC] = np.eye(BC)
    sel16 = sel16.astype(BF16)
    ident = np.eye(128, dtype=np.float32).astype(BF16)

    vp = NCORES * VC  # 10240
    fc_W_pad = np.zeros((vp, H), np.float32)
    fc_W_pad[:V] = fc_W
    fc_b_pad = np.zeros((vp,), np.float32)
    fc_b_pad[:V] = fc_b

    shared = {"whh": whh, "sel16": sel16, "ident": ident}

    per_core = []
    for c in range(NCORES):
        rows = slice(c * BC, (c + 1) * BC)
        xgt = np.concatenate([xgt_core, feat_xg[rows]], axis=0)  # [VAUG, 4096]
        idx = np.zeros((BC, T), np.int32)
        idx[:, 0] = V + np.arange(BC)
        idx[:, 1:] = captions[rows, 1:T].astype(np.int32)
        wslice = fc_W_pad[c * VC : (c + 1) * VC]  # [1280, 1024]
        fcw = np.ascontiguousarray(
            wslice.reshape(NVT, 128, 8, 128).transpose(0, 2, 3, 1)
        ).astype(BF16)  # [v, kc, k, j]
        fcb = np.ascontiguousarray(
            fc_b_pad[c * VC : (c + 1) * VC].reshape(NVT, 128).T
        ).astype(np.float32)  # [128, NVT]
        per_core.append({"xgt": xgt, "idx": idx, "fcw": fcw, "fcb": fcb})
    return shared, per_core


_NC_CACHE = {}


def kernel(features, captions, embed_W, W_ih, W_hh, b_ih, b_hh, fc_W, fc_b):
    from concourse.bass_utils import run_bass_kernel_spmd

    features = np.asarray(features)
    captions = np.asarray(captions)
    embed_W = np.asarray(embed_W)
    W_ih = np.asarray(W_ih)
    W_hh = np.asarray(W_hh)
    b_ih = np.asarray(b_ih)
    b_hh = np.asarray(b_hh)
    fc_W = np.asarray(fc_W)
    fc_b = np.asarray(fc_b)

    if "nc" not in _NC_CACHE:
        _NC_CACHE["nc"] = build_nc()
    nc = _NC_CACHE["nc"]

    shared, per_core = prep_host(
        features, captions, embed_W, W_ih, W_hh, b_ih, b_hh, fc_W, fc_b
    )
    in_maps = [{**shared, **pc} for pc in per_core]
    res = run_bass_kernel_spmd(nc, in_maps, list(range(NCORES)))
    _NC_CACHE["last_results"] = res
    _NC_CACHE["last_in_maps"] = in_maps

    # out_lg: [NVT, 128, src_core, block, S*BC] -> out[b, t, v]
    out = np.empty((B, T, V), np.float32)
    for c in range(NCORES):
        lg = res.results[c]["out_lg"]  # vocab rows [VC*c : VC*(c+1)]
        # [v1, p, k, b, s, j] -> batch 16k+j, step 8b+s, vocab 128*v1+p
        arr = lg.reshape(NVT, 128, NCORES, NB, S, BC)
        arr = arr.transpose(2, 5, 3, 4, 0, 1).reshape(B, T, VC)
        nv = min(VC, V - c * VC)
        if nv > 0:
            out[:, :, c * VC : c * VC + nv] = arr[:, :, :nv]
    return out


# revision 26
# speedup vs baseline: 24.3335x; 24.3335x over previous
"""Trainium2 Bass kernel for nn_Decoder (LSTM decoder: embed -> LSTM -> vocab fc).

Strategy (v2):
  - Host folds embedding + input projection + biases into one gather table:
    xg_table = embed_W @ W_ih^T + b_ih + b_hh  (gate-column-permuted). Per
    step the kernel indirect-DMA-gathers 16 rows -> no device-side embedding
    transposes, no xg matmuls, no bias matmuls.
  - Recurrence is data-parallel over batch (16 rows/core), 4-way PE column
    tiling for the h @ W_hh^T matmul (as before). h^T is produced by 2 PE
    transposes + 8 narrow DVE copies (instead of 32 DVE transposes).
  - Every S=8 steps, the per-core h^T block is AllGather'd (DRAM->DRAM) so
    every core holds h^T for the FULL batch; the fc projection is sharded
    over vocab (1280 rows/core) with its weights RESIDENT in SBUF, and its
    matmuls (full-array, N=512) are interleaved into the recurrence's PE
    gaps.
"""

import sys

sys.path.insert(0, "/opt/trn_rl_repo")

import numpy as np
import ml_dtypes

import concourse.bass as bass
import concourse.bacc as bacc
import concourse.mybir as mybir
import concourse.tile as tile

BF16 = ml_dtypes.bfloat16

# Problem shapes
B, T, E, H, V = 128, 64, 512, 1024, 10000
NCORES = 8
BC = B // NCORES        # 16 batch rows per core
G = 4                   # PE column-tile groups for the recurrence
S = 8                   # steps per h^T block (AllGather granularity)
NB = T // S             # 8 blocks
VC = 1280               # vocab rows per core (10240 padded / 8)
NVT = VC // 128         # 10 vocab tiles per core
VAUG = V + BC           # xg table rows: vocab + per-core feature rows
LAG = 4                 # steps between block boundary and fc eligibility
FC_A = 1                # fc units emitted after the h-matmuls
FC_B = 1                # fc units emitted after the transposes

F32 = mybir.dt.float32
BF = mybir.dt.bfloat16
I32 = mybir.dt.int32


def build_nc():
    nc = bacc.Bacc("TRN2", num_devices=NCORES)

    xgt_d = nc.declare_dram_parameter("xgt", [VAUG, 4 * H], BF, isOutput=False)
    idx_d = nc.declare_dram_parameter("idx", [BC, T], I32, isOutput=False)
    whh_d = nc.declare_dram_parameter("whh", [G, 8, 128, 1024], BF, isOutput=False)
    sel_d = nc.declare_dram_parameter("sel16", [128, BC], BF, isOutput=False)
    id_d = nc.declare_dram_parameter("ident", [128, 128], BF, isOutput=False)
    fcw_d = nc.declare_dram_parameter("fcw", [NVT, 8, 128, 128], BF, isOutput=False)
    fcb_d = nc.declare_dram_parameter("fcb", [128, NVT], F32, isOutput=False)
    # out layout: (vtile, vpart, src_core, block, s*16+j)
    out_d = nc.declare_dram_parameter(
        "out_lg", [NVT, 128, NCORES, NB, S * BC], F32, isOutput=True
    )

    # partition-major staging so gathered blocks load with one DMA per core
    hsb_in_d = nc.dram_tensor("hsb_in", [NB, 128, 8, S * BC], BF)
    hsb_out_d = nc.dram_tensor(
        "hsb_out", [NB, NCORES, 128, 8, S * BC], BF, addr_space="Shared"
    )

    XB = 4  # xgm prefetch depth

    with tile.TileContext(nc) as tc:
        with (
            tc.tile_pool(name="persist", bufs=1) as pp,
            tc.tile_pool(name="gates", bufs=3) as gates_p,
            tc.tile_pool(name="ew", bufs=3) as ew_p,
            tc.tile_pool(name="logit", bufs=6) as logit_p,
            tc.tile_pool(name="gpsum", bufs=1, space="PSUM") as gps_p,
            tc.tile_pool(name="fpsum", bufs=3, space="PSUM") as fps_p,
            tc.tile_pool(name="tpsum", bufs=1, space="PSUM") as tps_p,
        ):
            # ---- small persistent tiles first (cheap DMAs, unblock step 0) ----
            idx_sb = pp.tile([BC, T], I32, tag="idx")
            nc.sync.dma_start(out=idx_sb[:, :], in_=idx_d[:, :])
            sel_sb = pp.tile([128, BC], BF, tag="sel16")
            nc.sync.dma_start(out=sel_sb[:, :], in_=sel_d[:, :])
            id_sb = pp.tile([128, 128], BF, tag="ident")
            nc.sync.dma_start(out=id_sb[:, :], in_=id_d[:, :])
            fcb_sb = pp.tile([128, NVT], F32, tag="fcb")
            nc.sync.dma_start(out=fcb_sb[:, :], in_=fcb_d[:, :])

            # xgm ring: rows 0:16 hold gathered xg rows; junk rows stay 0
            xgm = []
            for par in range(XB):
                xt = pp.tile([128, 4 * H], BF, tag=f"xgm_{par}", name=f"xgm_{par}")
                nc.vector.memset(xt[:, :], 0.0)
                xgm.append(xt)

            def gather_xg(t):
                nc.gpsimd.indirect_dma_start(
                    out=xgm[t % XB][0:BC, :],
                    out_offset=None,
                    in_=xgt_d[:, :],
                    in_offset=bass.IndirectOffsetOnAxis(
                        ap=idx_sb[:, t : t + 1], axis=0
                    ),
                )

            for t in range(min(XB - 1, T)):
                gather_xg(t)

            # ---- weights ----
            whh_sb = {}
            for k in range(8):
                for g in range(G):
                    w = pp.tile([128, 1024], BF, tag=f"w_{g}_{k}", name=f"w_{g}_{k}")
                    nc.sync.dma_start(out=w[:, :], in_=whh_d[g, k, :, :])
                    whh_sb[(g, k)] = w
            fcw_sb = []
            for v in range(NVT):
                fw = pp.tile([128, 1024], BF, tag=f"fcw_{v}", name=f"fcw_{v}")
                for kc in range(8):
                    nc.sync.dma_start(
                        out=fw[:, 128 * kc : 128 * kc + 128], in_=fcw_d[v, kc, :, :]
                    )
                fcw_sb.append(fw)

            # ---- state tiles ----
            # h^T accumulation ring: 2 block slots of S*BC=128 cols per chunk
            hsT = [
                pp.tile([128, 2 * S * BC], BF, tag=f"hsT_{kc}", name=f"hsT_{kc}")
                for kc in range(8)
            ]
            # gathered full-batch h^T ring: 2 block slots, kc-major columns
            hfull = [
                pp.tile([128, 8, S * B], BF, tag=f"hf_{sl}", name=f"hf_{sl}")
                for sl in range(2)
            ]
            c_sb = pp.tile([128, 256], F32, tag="c_state")

            gps = [
                gps_p.tile([128, 1024], F32, tag="gps0", name="gps0"),
                gps_p.tile([128, 1024], F32, tag="gps1", name="gps1"),
            ]
            nc.vector.memset(gps[0][:, :], 0.0)
            nc.vector.memset(gps[1][:, :], 0.0)

            # ---- fc emission ----
            fc_queue = []  # (block, vtile, half) eligible units

            evict_list = []  # fc units whose matmuls are issued, eviction pending

            def emit_fc_mms(n):
                """Issue the PE matmuls for up to n queued fc units (one unit =
                both 512-token halves of a (block, vtile), sharing each
                stationary load); defer the ACT eviction so it never precedes
                the step's gate activations in the in-order ACT queue."""
                for _ in range(min(n, len(fc_queue))):
                    b, v = fc_queue.pop(0)
                    sl = b % 2
                    fps = [
                        fps_p.tile([128, 512], F32, tag="fps", name=f"fps{i}")
                        for i in range(2)
                    ]
                    for kc in range(8):
                        for hf in range(2):
                            nc.tensor.matmul(
                                fps[hf][:, :],
                                fcw_sb[v][:, 128 * kc : 128 * kc + 128],
                                hfull[sl][:, kc, 512 * hf : 512 * hf + 512],
                                start=(kc == 0),
                                stop=(kc == 7),
                                skip_group_check=True,
                            )
                    for hf in range(2):
                        evict_list.append((fps[hf], b, v, hf))

            def flush_evicts():
                while evict_list:
                    fps, b, v, hf = evict_list.pop(0)
                    lg = logit_p.tile([128, 512], F32, tag="lg")
                    nc.scalar.activation(
                        lg[:, :],
                        fps[:, :],
                        mybir.ActivationFunctionType.Identity,
                        bias=fcb_sb[:, v : v + 1],
                    )
                    nc.sync.dma_start(
                        out=out_d[v, :, 4 * hf : 4 * hf + 4, b, :], in_=lg[:, :]
                    )

            pending = []  # blocks gathered but not yet eligible: (block, ready_t)

            def release_pending(t):
                while pending and pending[0][1] <= t:
                    b, _ = pending.pop(0)
                    for v in range(NVT):
                        fc_queue.append((b, v))

            # ---- recurrence ----
            for t in range(T):
                ps = gps[t % 2]
                release_pending(t)
                if t + XB - 1 < T:
                    gather_xg(t + XB - 1)

                # gate matmuls: xg injection first (independent of h(t-1)),
                # then the 8 h-chunk contributions
                nks = 1 if t == 0 else 9
                for half in range(2):
                    cs = slice(512 * half, 512 * half + 512)
                    for g in range(G):
                        nc.tensor.matmul(
                            ps[32 * g : 32 * g + BC, cs],
                            sel_sb[:, :],
                            xgm[t % XB][:, 1024 * g + 512 * half :][:, 0:512],
                            start=True,
                            stop=(nks == 1),
                            tile_position=(0, 32 * g),
                            skip_group_check=True,
                        )
                if t > 0:
                    pc = ((t - 1) // S) % 2 * (S * BC) + ((t - 1) % S) * BC
                    for ki in range(8):
                        for half in range(2):
                            cs = slice(512 * half, 512 * half + 512)
                            for g in range(G):
                                nc.tensor.matmul(
                                    ps[32 * g : 32 * g + BC, cs],
                                    hsT[ki][:, pc : pc + BC],
                                    whh_sb[(g, ki)][:, cs],
                                    start=False,
                                    stop=(ki == 7),
                                    tile_position=(0, 32 * g),
                                    skip_group_check=True,
                                )

                emit_fc_mms(FC_A)

                # elementwise: gate order per group is [i | f | o | g]
                gt = gates_p.tile([128, 1024], F32, tag="gt")
                nc.scalar.activation(
                    gt[:, 0:768], ps[:, 0:768], mybir.ActivationFunctionType.Sigmoid
                )
                nc.scalar.activation(
                    gt[:, 768:1024], ps[:, 768:1024], mybir.ActivationFunctionType.Tanh
                )
                if t == 0:
                    nc.vector.tensor_mul(c_sb[:, :], gt[:, 0:256], gt[:, 768:1024])
                else:
                    tmp1 = ew_p.tile([128, 256], F32, tag="tmp1")
                    nc.vector.tensor_mul(tmp1[:, :], gt[:, 0:256], gt[:, 768:1024])
                    nc.vector.tensor_mul(c_sb[:, :], gt[:, 256:512], c_sb[:, :])
                    nc.vector.tensor_add(c_sb[:, :], c_sb[:, :], tmp1[:, :])
                tcs = ew_p.tile([128, 256], F32, tag="tcs")
                nc.scalar.activation(
                    tcs[:, :], c_sb[:, :], mybir.ActivationFunctionType.Tanh
                )
                h_sb = ew_p.tile([128, 256], BF, tag="h_sb")
                nc.vector.tensor_mul(h_sb[:, :], gt[:, 512:768], tcs[:, :])

                flush_evicts()

                # h -> h^T: 2 PE transposes + 8 narrow copies
                cc = (t // S) % 2 * (S * BC) + (t % S) * BC
                for gam in range(2):
                    tps = tps_p.tile([128, 128], BF, tag="tps")
                    nc.tensor.transpose(
                        tps[:, :], h_sb[:, 128 * gam : 128 * gam + 128], id_sb[:, :]
                    )
                    for g in range(G):
                        nc.vector.tensor_copy(
                            hsT[2 * g + gam][:, cc : cc + BC],
                            tps[:, 32 * g : 32 * g + BC],
                        )

                emit_fc_mms(FC_B)
                flush_evicts()

                # block boundary: stage own h^T block, AllGather, load gathered
                if (t + 1) % S == 0:
                    b = t // S
                    sl = b % 2
                    for kc in range(8):
                        nc.sync.dma_start(
                            out=hsb_in_d[b, :, kc, :],
                            in_=hsT[kc][:, sl * S * BC : (sl + 1) * S * BC],
                        )
                    nc.gpsimd.collective_compute(
                        "AllGather",
                        mybir.AluOpType.bypass,
                        replica_groups=[list(range(NCORES))],
                        ins=[hsb_in_d[b, :, :, :]],
                        outs=[hsb_out_d[b, :, :, :, :]],
                    )
                    for core in range(NCORES):
                        nc.gpsimd.dma_start(
                            out=hfull[sl][:, :, 128 * core : 128 * core + 128],
                            in_=hsb_out_d[b, core, :, :, :],
                        )
                    pending.append((b, t + 1 + LAG))

            # ---- epilogue: drain remaining fc work ----
            release_pending(10**9)
            while fc_queue:
                emit_fc_mms(1)
                flush_evicts()

    nc.finalize()
    return nc


def prep_host(features, captions, embed_W, W_ih, W_hh, b_ih, b_hh, fc_W, fc_b):
    """Host-side layout prep. Returns (shared dict, per-core list)."""
    # gate-column permutation: group g holds H-range [256g:256g+256) of each
    # gate, column order within group = [i | f | o | gg] (256 each)
    sec_base = np.array([0, H, 3 * H, 2 * H])
    j = np.arange(1024)
    perm = np.empty((G, 1024), np.int64)
    for g in range(G):
        perm[g] = sec_base[j // 256] + 256 * g + (j % 256)
    full_perm = perm.reshape(-1)  # [4096] column order: group-major

    bias = (b_ih + b_hh).astype(np.float32)

    # xg gather table: (embed @ W_ih^T + bias), columns permuted
    xgt_core = (embed_W.astype(np.float32) @ W_ih.T.astype(np.float32)) + bias
    xgt_core = xgt_core[:, full_perm].astype(BF16)  # [V, 4096]
    feat_xg = (features.astype(np.float32) @ W_ih.T.astype(np.float32)) + bias
    feat_xg = feat_xg[:, full_perm].astype(BF16)  # [B, 4096]

    whh = np.zeros((G, 8, 128, 1024), np.float32)
    for g in range(G):
        selw = W_hh[perm[g]]  # [1024 gate-cols, 1024]
        for k in range(8):
            whh[g, k] = selw[:, 128 * k : 128 * k + 128].T
    whh = whh.astype(BF16)

    sel16 = np.zeros((128, BC), np.float32)
    sel16[:BC, :BC] = np.eye(BC)
    sel16 = sel16.astype(BF16)
    ident = np.eye(128, dtype=np.float32).astype(BF16)

    vp = NCORES * VC  # 10240
    fc_W_pad = np.zeros((vp, H), np.float32)
    fc_W_pad[:V] = fc_W
    fc_b_pad = np.zeros((vp,), np.float32)
    fc_b_pad[:V] = fc_b

    shared = {"whh": whh, "sel16": sel16, "ident": ident}

    per_core = []
    for c in range(NCORES):
        rows = slice(c * BC, (c + 1) * BC)
        xgt = np.concatenate([xgt_core, feat_xg[rows]], axis=0)  # [VAUG, 4096]
        idx = np.zeros((BC, T), np.int32)
        idx[:, 0] = V + np.arange(BC)
        idx[:, 1:] = captions[rows, 1:T].astype(np.int32)
        wslice = fc_W_pad[c * VC : (c + 1) * VC]  # [1280, 1024]
        fcw = np.ascontiguousarray(
            wslice.reshape(NVT, 128, 8, 128).transpose(0, 2, 3, 1)
        ).astype(BF16)  # [v, kc, k, j]
        fcb = np.ascontiguousarray(
            fc_b_pad[c * VC : (c + 1) * VC].reshape(NVT, 128).T
        ).astype(np.float32)  # [128, NVT]
        per_core.append({"xgt": xgt, "idx": idx, "fcw": fcw, "fcb": fcb})
    return shared, per_core


_NC_CACHE = {}


def kernel(features, captions, embed_W, W_ih, W_hh, b_ih, b_hh, fc_W, fc_b):
    from concourse.bass_utils import run_bass_kernel_spmd

    features = np.asarray(features)
    captions = np.asarray(captions)
    embed_W = np.asarray(embed_W)
    W_ih = np.asarray(W_ih)
    W_hh = np.asarray(W_hh)
    b_ih = np.asarray(b_ih)
    b_hh = np.asarray(b_hh)
    fc_W = np.asarray(fc_W)
    fc_b = np.asarray(fc_b)

    if "nc" not in _NC_CACHE:
        _NC_CACHE["nc"] = build_nc()
    nc = _NC_CACHE["nc"]

    shared, per_core = prep_host(
        features, captions, embed_W, W_ih, W_hh, b_ih, b_hh, fc_W, fc_b
    )
    in_maps = [{**shared, **pc} for pc in per_core]
    res = run_bass_kernel_spmd(nc, in_maps, list(range(NCORES)))
    _NC_CACHE["last_results"] = res
    _NC_CACHE["last_in_maps"] = in_maps

    # out_lg: [NVT, 128, src_core, block, S*BC] -> out[b, t, v]
    out = np.empty((B, T, V), np.float32)
    for c in range(NCORES):
        lg = res.results[c]["out_lg"]  # vocab rows [VC*c : VC*(c+1)]
        # [v1, p, k, b, s, j] -> batch 16k+j, step 8b+s, vocab 128*v1+p
        arr = lg.reshape(NVT, 128, NCORES, NB, S, BC)
        arr = arr.transpose(2, 5, 3, 4, 0, 1).reshape(B, T, VC)
        nv = min(VC, V - c * VC)
        if nv > 0:
            out[:, :, c * VC : c * VC + nv] = arr[:, :, :nv]
    return out
